# revision 14
# baseline (speedup 1.0000x reference)
"""ArcFace loss on 8 TRN2 NeuronCores (Bass/Tile).

Strategy (model-parallel classification head, device kernel = pure
matmul+exp stream):
  - Host: l2-normalize embeddings and weights (fp32), quantize to fp8
    (e_hat*32, w_hat*128), shard classes across 8 cores (12500/core,
    zero-padded to 12544), and compute the per-row target-class
    corrections (ArcFace margin) in float64 from the fp32 inputs.
  - Device (per core): cosine slice = e_hat @ w_hat_local^T on the
    TensorEngine (fp8 DoubleRow), then sum_c exp(64*cos) per 2048-col
    PSUM group, spread over up to three drain engines so the PE never
    stalls (a stall-free PE stream also lets the clock ramp MID->MAX):
      * ACT: exp activation with accum_out (exact, scale 2^-6).
      * DVE/Pool: Schraudolph bf16 exp -- i16 = f32_to_i16(psm*a + b)
        is a piecewise-linear log2 approximation whose bit pattern IS
        exp(64*cos) in bf16; a second pass sums the bitcast values
        (DVE: tensor_scalar+accum, Pool: reduce_sum). ~0.3% rms /
        ~0.2% mean error per group sum, correctable on host (slots
        are output separately).
  - Host: S[b] = sum over cores/groups of the slot sums (minus the
    zero-pad columns that contribute exp(0)=1 each), swap in the margin
    target term, loss = mean(log(S') - l_m). No on-device collectives:
    the cross-core reduction is 8 x 28KB, cheaper on host than a ~29us
    mesh AllReduce.

kernel(**inputs) takes the FULL inputs and returns the full (scalar)
output.
"""

import math

import numpy as np
import ml_dtypes

import concourse.bass as bass
import concourse.mybir as mybir
import concourse.tile as tile
from concourse import bacc

AF = mybir.ActivationFunctionType
ALU = mybir.AluOpType
AX = mybir.AxisListType
F32 = mybir.dt.float32
BF16 = mybir.dt.bfloat16
I16 = mybir.dt.int16
FP8 = mybir.dt.float8e4

MARGIN = 0.5
SCALE = 64.0
EPS = 1e-7

S_E = 32.0
S_W = 128.0

# bf16 Schraudolph: bits(exp(t)) ~ 128*(t*log2(e) + 127) - C,  C ~= 7
A_SCH = 128.0 * math.log2(math.e) * (SCALE / (S_E * S_W))
B_SCH = 16256.0 - 7.0
# host-side divisor for the systematic Schraudolph bias on DVE/Pool slots
# (measured +0.22% on HW with round-to-nearest f32->i16 conversion)
SCH_BIAS = 0.0022

# drain-engine pattern over full-width (2048) tiles, cycled.
# 2 A : 1 D balances ACT (2.25us/tile) against DVE (4.57us/tile);
# GPSIMD can neither read PSUM nor run reduce/accum ops, so it is
# not usable as a third drain engine.
DRAIN_PATTERN = "ADA"


def make_cfg(n_cores=8, b=1024, d=512, c_total=100000, pattern=None):
    c_local = c_total // n_cores
    c_pad = ((c_local + 127) // 128) * 128
    # first group small (pipeline fill starts after a 0.5MB DMA), last
    # groups small (short drain tail after the final matmul)
    grp_w = [1024, 2048, 2048, 2048, 2048, 2048, 1024, 256]
    assert sum(grp_w) == c_pad
    return dict(
        n_cores=n_cores,
        b=b,
        d=d,
        c_total=c_total,
        c_local=c_local,
        c_pad=c_pad,
        grp_w=grp_w,
        pattern=pattern or DRAIN_PATTERN,
    )


def _drains(cfg):
    """Map (gi, bo) -> 'A' | 'D' drain engine."""
    out = {}
    pat = cfg["pattern"]
    t = 0
    for gi, gw in enumerate(cfg["grp_w"]):
        for bo in range(cfg["b"] // 128):
            if gw < 1024:
                out[(gi, bo)] = "A"
            else:
                out[(gi, bo)] = pat[t % len(pat)]
                t += 1
    return out


def build_nc(cfg):
    n_cores = cfg["n_cores"]
    b, d = cfg["b"], cfg["d"]
    c_pad = cfg["c_pad"]
    grp_w = cfg["grp_w"]
    NG = len(grp_w)
    grp_off = [0]
    for gw in grp_w:
        grp_off.append(grp_off[-1] + gw)
    KO = d // 128
    BO = b // 128
    P = 128
    drains = _drains(cfg)

    nc = bacc.Bacc(
        "TRN2",
        target_bir_lowering=False,
        debug=False,
        enable_asserts=True,
        num_devices=n_cores,
    )

    wt_d = nc.dram_tensor("wt", [P, KO * c_pad], FP8, kind="ExternalInput")
    et_d = nc.dram_tensor("et", [P, KO * b], FP8, kind="ExternalInput")
    out_d = nc.dram_tensor("out", [P, BO * NG], F32, kind="ExternalOutput")

    with tile.TileContext(nc) as tc:
        with (
            tc.tile_pool(name="big", bufs=1) as pb,
            tc.tile_pool(name="wpool", bufs=NG) as pw,
            tc.tile_pool(name="scr", bufs=2) as pscr,
            tc.tile_pool(name="i16p", bufs=2) as pi16,
            tc.tile_pool(name="jnkp", bufs=2) as pjnk,
            tc.tile_pool(name="small", bufs=1) as ps,
            tc.tile_pool(name="ps_all", bufs=2, space="PSUM") as pps,
        ):
            # ---- load replicated embeddings (scalar queue: the ACT
            # engine is idle until the first exp, ~7us later). Split by
            # k-half so the kp=0 matmuls can start after 0.25MB lands ----
            et_sb = pb.tile([P, KO, b], FP8, tag="et")
            et_src = et_d.ap().rearrange("p (k b) -> p k b", k=KO)
            nc.scalar.dma_start(et_sb[:, 0:2, :], et_src[:, 0:2, :])
            nc.scalar.dma_start(et_sb[:, 2:4, :], et_src[:, 2:4, :])
            # ---- prefetch all weight groups across two DMA queues
            # (each group block is contiguous in DRAM) ----
            w_tiles = []
            for gi in range(NG):
                gw = grp_w[gi]
                c0 = grp_off[gi]
                Wg = pw.tile([P, KO, gw], FP8, tag=f"Wg{gi}", bufs=1)
                w_tiles.append(Wg)
                src = wt_d.ap()[:, KO * c0 : KO * (c0 + gw)].rearrange(
                    "p (k n) -> p k n", k=KO
                )
                if gi % 2 == 0:
                    nc.sync.dma_start(Wg[:, :, :gw], src)
                else:
                    nc.gpsimd.dma_start(Wg[:, :, :gw], src)

            sums = ps.tile([P, BO, NG], F32, tag="sums")
            for gi in range(NG):
                gw = grp_w[gi]
                Wg = w_tiles[gi]
                for bo in range(BO):
                    bs = slice(bo * P, (bo + 1) * P)
                    psm = pps.tile([P, 2048], F32, tag="ps")
                    for kp in range(KO // 2):
                        ks = slice(2 * kp, 2 * kp + 2)
                        for o in range(0, gw, 512):
                            nw = min(512, gw - o)
                            nc.tensor.matmul(
                                psm[:, o : o + nw],
                                et_sb[:, ks, bs],
                                Wg[:, ks, o : o + nw],
                                start=(kp == 0),
                                stop=(kp == KO // 2 - 1),
                                perf_mode=mybir.MatmulPerfMode.DoubleRow,
                            )
                    drain = drains[(gi, bo)]
                    slot = sums[:, bo, gi : gi + 1]
                    if drain == "A":
                        scr = pscr.tile([P, 2048], BF16, tag="escr")
                        nc.scalar.activation(
                            scr[:, :gw],
                            psm[:, :gw],
                            AF.Exp,
                            scale=SCALE / (S_E * S_W),
                            accum_out=slot,
                        )
                    elif drain == "D":
                        i16_t = pi16.tile([P, 2048], I16, tag="i16")
                        nc.vector.tensor_scalar(
                            i16_t[:, :gw], psm[:, :gw],
                            A_SCH, B_SCH, ALU.mult, ALU.add,
                        )
                        jnk = pjnk.tile([P, 2048], BF16, tag="jnk")
                        nc.vector.tensor_scalar(
                            jnk[:, :gw], i16_t[:, :gw].bitcast(BF16),
                            1.0, 0.0, ALU.mult, ALU.add,
                            accum_out=slot,
                        )
                    else:  # 'P'
                        i16_t = pi16.tile([P, 2048], I16, tag="i16g")
                        nc.gpsimd.tensor_scalar(
                            i16_t[:, :gw], psm[:, :gw],
                            A_SCH, B_SCH, ALU.mult, ALU.add,
                        )
                        nc.gpsimd.reduce_sum(
                            slot, i16_t[:, :gw].bitcast(BF16), axis=AX.X
                        )

            nc.sync.dma_start(
                out_d.ap(), sums[:].rearrange("p b g -> p (b g)")
            )

    nc.compile()
    return nc


def prep_inputs(cfg, embeddings, weight):
    """Normalize + quantize + shard the full inputs into per-core in_maps."""
    n_cores = cfg["n_cores"]
    b, d = cfg["b"], cfg["d"]
    c_local, c_pad = cfg["c_local"], cfg["c_pad"]
    KO = d // 128
    P = 128

    e = np.asarray(embeddings, np.float32)
    w = np.asarray(weight, np.float32)
    e_hat = e / np.maximum(
        np.linalg.norm(e, axis=-1, keepdims=True), 1e-12
    )
    w_hat = w / np.maximum(
        np.linalg.norm(w, axis=-1, keepdims=True), 1e-12
    )

    et = (e_hat.T * S_E).astype(ml_dtypes.float8_e4m3)
    et_host = np.ascontiguousarray(
        et.reshape(KO, P, b).transpose(1, 0, 2).reshape(P, KO * b)
    )

    in_maps = []
    for i in range(n_cores):
        ws = w_hat[i * c_local : (i + 1) * c_local]
        if c_pad > c_local:
            ws = np.concatenate(
                [ws, np.zeros((c_pad - c_local, d), np.float32)], axis=0
            )
        wt = (ws * S_W).astype(ml_dtypes.float8_e4m3).T  # [d, c_pad]
        wt4 = np.ascontiguousarray(wt).reshape(KO, P, c_pad)
        blocks = []
        c0 = 0
        for gw in cfg["grp_w"]:
            blk = wt4[:, :, c0 : c0 + gw]  # [KO, P, gw]
            blocks.append(blk.transpose(1, 0, 2).reshape(P, KO * gw))
            c0 += gw
        wt_host = np.ascontiguousarray(np.concatenate(blocks, axis=1))
        in_maps.append({"wt": wt_host, "et": et_host})
    return in_maps, e_hat, w_hat


_CACHED = {}


def _get_nc(cfg_key, cfg):
    if cfg_key not in _CACHED:
        _CACHED[cfg_key] = build_nc(cfg)
    return _CACHED[cfg_key]


def run(inputs, mm_dtype="fp8", trace=False, **kw):
    from concourse.bass_utils import run_bass_kernel_spmd

    cfg = make_cfg()
    nc = _get_nc((mm_dtype,), cfg)
    in_maps, e_hat, w_hat = prep_inputs(
        cfg, inputs["embeddings"], inputs["weight"]
    )
    res = run_bass_kernel_spmd(
        nc, in_maps, core_ids=list(range(cfg["n_cores"])), trace=trace, **kw
    )

    b = cfg["b"]
    BO = b // 128
    grp_w = cfg["grp_w"]
    NG = len(grp_w)
    drains = _drains(cfg)
    # per-slot correction for the Schraudolph bias on DVE/Pool groups
    corr = np.ones((BO, NG), np.float64)
    for gi in range(NG):
        for bo in range(BO):
            if drains[(gi, bo)] != "A":
                corr[bo, gi] = 1.0 / (1.0 + SCH_BIAS)
    # slot[p, bo, gi] holds rows b = bo*128 + p
    S = np.zeros(b, np.float64)
    for i in range(cfg["n_cores"]):
        slots = (
            res.results[i]["out"].astype(np.float64).reshape(128, BO, NG)
        )
        slots = slots * corr[None, :, :]
        S += slots.sum(axis=2).T.reshape(-1)
    # each core's (c_pad - c_local) zero-pad columns sit in the last
    # (ACT-drained) group and contribute exp(0) = 1 exactly
    S -= float(cfg["n_cores"] * (cfg["c_pad"] - cfg["c_local"]))

    labels = np.asarray(inputs["labels"]).astype(np.int64)
    cos_t = np.einsum(
        "bd,bd->b",
        e_hat.astype(np.float64),
        w_hat[labels].astype(np.float64),
    )
    cos_c = np.clip(cos_t, -1.0 + EPS, 1.0 - EPS)
    theta = np.arccos(cos_c)
    l_t = SCALE * cos_t
    l_m = SCALE * np.cos(theta + MARGIN)
    S2 = S - np.exp(l_t) + np.exp(l_m)
    loss = np.mean(np.log(S2) - l_m)
    return np.float32(loss), res


def kernel(**inputs):
    loss, _ = run(inputs, trace=False)
    return np.asarray(loss, dtype=np.float32).reshape(())


# revision 15
# speedup vs baseline: 1.0546x; 1.0546x over previous
"""ArcFace loss on 8 TRN2 NeuronCores (Bass/Tile).

Strategy (model-parallel classification head, device kernel = pure
matmul+exp stream):
  - Host: l2-normalize embeddings and weights (fp32), quantize to fp8
    (e_hat*32, w_hat*128), shard classes across 8 cores (12500/core,
    zero-padded to 12544), and compute the per-row target-class
    corrections (ArcFace margin) in float64 from the fp32 inputs.
  - Device (per core): cosine slice = e_hat @ w_hat_local^T on the
    TensorEngine (fp8 DoubleRow), then sum_c exp(64*cos) per 2048-col
    PSUM group, spread over up to three drain engines so the PE never
    stalls (a stall-free PE stream also lets the clock ramp MID->MAX):
      * ACT: exp activation with accum_out (exact, scale 2^-6).
      * DVE/Pool: Schraudolph bf16 exp -- i16 = f32_to_i16(psm*a + b)
        is a piecewise-linear log2 approximation whose bit pattern IS
        exp(64*cos) in bf16; a second pass sums the bitcast values
        (DVE: tensor_scalar+accum, Pool: reduce_sum). ~0.3% rms /
        ~0.2% mean error per group sum, correctable on host (slots
        are output separately).
  - Host: S[b] = sum over cores/groups of the slot sums (minus the
    zero-pad columns that contribute exp(0)=1 each), swap in the margin
    target term, loss = mean(log(S') - l_m). No on-device collectives:
    the cross-core reduction is 8 x 28KB, cheaper on host than a ~29us
    mesh AllReduce.

kernel(**inputs) takes the FULL inputs and returns the full (scalar)
output.
"""

import math

import numpy as np
import ml_dtypes

import concourse.bass as bass
import concourse.mybir as mybir
import concourse.tile as tile
from concourse import bacc

AF = mybir.ActivationFunctionType
ALU = mybir.AluOpType
AX = mybir.AxisListType
F32 = mybir.dt.float32
BF16 = mybir.dt.bfloat16
I16 = mybir.dt.int16
FP8 = mybir.dt.float8e4

MARGIN = 0.5
SCALE = 64.0
EPS = 1e-7

S_E = 32.0
S_W = 128.0

# bf16 Schraudolph: bits(exp(t)) ~ 128*(t*log2(e) + 127) - C,  C ~= 7
A_SCH = 128.0 * math.log2(math.e) * (SCALE / (S_E * S_W))
B_SCH = 16256.0 - 7.0
# host-side divisor for the systematic Schraudolph bias on DVE/Pool slots
# (measured +0.22% on HW with round-to-nearest f32->i16 conversion)
SCH_BIAS = 0.0022

# drain-engine pattern over full-width (2048) tiles, cycled.
# ACT costs ~2.25us/tile, the DVE Schraudolph path ~4.6us/tile; a
# saturated single-drain (all-ACT) pipeline measures faster than
# balanced multi-drain splits, which pay a PSUM-coupling stall tax.
# GPSIMD can neither read PSUM nor run reduce/accum ops.
DRAIN_PATTERN = "A"


def make_cfg(n_cores=8, b=1024, d=512, c_total=100000, pattern=None):
    c_local = c_total // n_cores
    c_pad = ((c_local + 127) // 128) * 128
    grp_w = [2048, 2048, 2048, 2048, 2048, 2048, 256]
    assert sum(grp_w) == c_pad
    return dict(
        n_cores=n_cores,
        b=b,
        d=d,
        c_total=c_total,
        c_local=c_local,
        c_pad=c_pad,
        grp_w=grp_w,
        pattern=pattern or DRAIN_PATTERN,
    )


def _drains(cfg):
    """Map (gi, bo) -> 'A' | 'D' drain engine."""
    out = {}
    pat = cfg["pattern"]
    t = 0
    for gi, gw in enumerate(cfg["grp_w"]):
        for bo in range(cfg["b"] // 128):
            if gw < 2048:
                out[(gi, bo)] = "A"
            else:
                out[(gi, bo)] = pat[t % len(pat)]
                t += 1
    return out


def build_nc(cfg):
    n_cores = cfg["n_cores"]
    b, d = cfg["b"], cfg["d"]
    c_pad = cfg["c_pad"]
    grp_w = cfg["grp_w"]
    NG = len(grp_w)
    grp_off = [0]
    for gw in grp_w:
        grp_off.append(grp_off[-1] + gw)
    KO = d // 128
    BO = b // 128
    P = 128
    drains = _drains(cfg)

    nc = bacc.Bacc(
        "TRN2",
        target_bir_lowering=False,
        debug=False,
        enable_asserts=True,
        num_devices=n_cores,
    )

    wt_d = nc.dram_tensor("wt", [P, KO * c_pad], FP8, kind="ExternalInput")
    et_d = nc.dram_tensor("et", [P, KO * b], FP8, kind="ExternalInput")
    out_d = nc.dram_tensor("out", [P, BO * NG], F32, kind="ExternalOutput")

    with tile.TileContext(nc) as tc:
        with (
            tc.tile_pool(name="big", bufs=1) as pb,
            tc.tile_pool(name="wpool", bufs=NG) as pw,
            tc.tile_pool(name="scr", bufs=2) as pscr,
            tc.tile_pool(name="i16p", bufs=2) as pi16,
            tc.tile_pool(name="jnkp", bufs=2) as pjnk,
            tc.tile_pool(name="small", bufs=1) as ps,
            tc.tile_pool(name="ps_all", bufs=2, space="PSUM") as pps,
        ):
            # ---- load replicated embeddings (scalar queue: the ACT
            # engine is idle until the first exp, ~7us later). Split by
            # k-half so the kp=0 matmuls can start after 0.25MB lands ----
            et_sb = pb.tile([P, KO, b], FP8, tag="et")
            et_src = et_d.ap().rearrange("p (k b) -> p k b", k=KO)
            nc.scalar.dma_start(et_sb[:, 0:2, :], et_src[:, 0:2, :])
            nc.scalar.dma_start(et_sb[:, 2:4, :], et_src[:, 2:4, :])
            # ---- prefetch all weight groups across two DMA queues
            # (each group block is contiguous in DRAM) ----
            w_tiles = []
            for gi in range(NG):
                gw = grp_w[gi]
                c0 = grp_off[gi]
                Wg = pw.tile([P, KO, gw], FP8, tag=f"Wg{gi}", bufs=1)
                w_tiles.append(Wg)
                src = wt_d.ap()[:, KO * c0 : KO * (c0 + gw)].rearrange(
                    "p (k n) -> p k n", k=KO
                )
                if gi == 0:
                    # k-major block: each k-half is contiguous in DRAM,
                    # and the kp=0 matmuls only need the first half
                    nc.sync.dma_start(Wg[:, 0:2, :], src[:, 0:2, :])
                    nc.sync.dma_start(Wg[:, 2:4, :], src[:, 2:4, :])
                elif gi % 2 == 0:
                    nc.sync.dma_start(Wg[:, :, :gw], src)
                else:
                    nc.gpsimd.dma_start(Wg[:, :, :gw], src)

            sums = ps.tile([P, BO, NG], F32, tag="sums")
            for gi in range(NG):
                gw = grp_w[gi]
                Wg = w_tiles[gi]
                for bo in range(BO):
                    bs = slice(bo * P, (bo + 1) * P)
                    psm = pps.tile([P, 2048], F32, tag="ps")
                    for kp in range(KO // 2):
                        ks = slice(2 * kp, 2 * kp + 2)
                        for o in range(0, gw, 512):
                            nw = min(512, gw - o)
                            nc.tensor.matmul(
                                psm[:, o : o + nw],
                                et_sb[:, ks, bs],
                                Wg[:, ks, o : o + nw],
                                start=(kp == 0),
                                stop=(kp == KO // 2 - 1),
                                perf_mode=mybir.MatmulPerfMode.DoubleRow,
                            )
                    drain = drains[(gi, bo)]
                    slot = sums[:, bo, gi : gi + 1]
                    if drain == "A":
                        scr = pscr.tile([P, 2048], BF16, tag="escr")
                        nc.scalar.activation(
                            scr[:, :gw],
                            psm[:, :gw],
                            AF.Exp,
                            scale=SCALE / (S_E * S_W),
                            accum_out=slot,
                        )
                    elif drain == "D":
                        i16_t = pi16.tile([P, 2048], I16, tag="i16")
                        nc.vector.tensor_scalar(
                            i16_t[:, :gw], psm[:, :gw],
                            A_SCH, B_SCH, ALU.mult, ALU.add,
                        )
                        jnk = pjnk.tile([P, 2048], BF16, tag="jnk")
                        nc.vector.tensor_scalar(
                            jnk[:, :gw], i16_t[:, :gw].bitcast(BF16),
                            1.0, 0.0, ALU.mult, ALU.add,
                            accum_out=slot,
                        )
                    else:  # 'P'
                        i16_t = pi16.tile([P, 2048], I16, tag="i16g")
                        nc.gpsimd.tensor_scalar(
                            i16_t[:, :gw], psm[:, :gw],
                            A_SCH, B_SCH, ALU.mult, ALU.add,
                        )
                        nc.gpsimd.reduce_sum(
                            slot, i16_t[:, :gw].bitcast(BF16), axis=AX.X
                        )

            nc.sync.dma_start(
                out_d.ap(), sums[:].rearrange("p b g -> p (b g)")
            )

    nc.compile()
    return nc


def prep_inputs(cfg, embeddings, weight):
    """Normalize + quantize + shard the full inputs into per-core in_maps."""
    n_cores = cfg["n_cores"]
    b, d = cfg["b"], cfg["d"]
    c_local, c_pad = cfg["c_local"], cfg["c_pad"]
    KO = d // 128
    P = 128

    e = np.asarray(embeddings, np.float32)
    w = np.asarray(weight, np.float32)
    e_hat = e / np.maximum(
        np.linalg.norm(e, axis=-1, keepdims=True), 1e-12
    )
    w_hat = w / np.maximum(
        np.linalg.norm(w, axis=-1, keepdims=True), 1e-12
    )

    et = (e_hat.T * S_E).astype(ml_dtypes.float8_e4m3)
    et_host = np.ascontiguousarray(
        et.reshape(KO, P, b).transpose(1, 0, 2).reshape(P, KO * b)
    )

    in_maps = []
    for i in range(n_cores):
        ws = w_hat[i * c_local : (i + 1) * c_local]
        if c_pad > c_local:
            ws = np.concatenate(
                [ws, np.zeros((c_pad - c_local, d), np.float32)], axis=0
            )
        wt = (ws * S_W).astype(ml_dtypes.float8_e4m3).T  # [d, c_pad]
        wt4 = np.ascontiguousarray(wt).reshape(KO, P, c_pad)
        blocks = []
        c0 = 0
        for gw in cfg["grp_w"]:
            blk = wt4[:, :, c0 : c0 + gw]  # [KO, P, gw]
            blocks.append(blk.transpose(1, 0, 2).reshape(P, KO * gw))
            c0 += gw
        wt_host = np.ascontiguousarray(np.concatenate(blocks, axis=1))
        in_maps.append({"wt": wt_host, "et": et_host})
    return in_maps, e_hat, w_hat


_CACHED = {}


def _get_nc(cfg_key, cfg):
    if cfg_key not in _CACHED:
        _CACHED[cfg_key] = build_nc(cfg)
    return _CACHED[cfg_key]


def run(inputs, mm_dtype="fp8", trace=False, **kw):
    from concourse.bass_utils import run_bass_kernel_spmd

    cfg = make_cfg()
    nc = _get_nc((mm_dtype,), cfg)
    in_maps, e_hat, w_hat = prep_inputs(
        cfg, inputs["embeddings"], inputs["weight"]
    )
    res = run_bass_kernel_spmd(
        nc, in_maps, core_ids=list(range(cfg["n_cores"])), trace=trace, **kw
    )

    b = cfg["b"]
    BO = b // 128
    grp_w = cfg["grp_w"]
    NG = len(grp_w)
    drains = _drains(cfg)
    # per-slot correction for the Schraudolph bias on DVE/Pool groups
    corr = np.ones((BO, NG), np.float64)
    for gi in range(NG):
        for bo in range(BO):
            if drains[(gi, bo)] != "A":
                corr[bo, gi] = 1.0 / (1.0 + SCH_BIAS)
    # slot[p, bo, gi] holds rows b = bo*128 + p
    S = np.zeros(b, np.float64)
    for i in range(cfg["n_cores"]):
        slots = (
            res.results[i]["out"].astype(np.float64).reshape(128, BO, NG)
        )
        slots = slots * corr[None, :, :]
        S += slots.sum(axis=2).T.reshape(-1)
    # each core's (c_pad - c_local) zero-pad columns sit in the last
    # (ACT-drained) group and contribute exp(0) = 1 exactly
    S -= float(cfg["n_cores"] * (cfg["c_pad"] - cfg["c_local"]))

    labels = np.asarray(inputs["labels"]).astype(np.int64)
    cos_t = np.einsum(
        "bd,bd->b",
        e_hat.astype(np.float64),
        w_hat[labels].astype(np.float64),
    )
    cos_c = np.clip(cos_t, -1.0 + EPS, 1.0 - EPS)
    theta = np.arccos(cos_c)
    l_t = SCALE * cos_t
    l_m = SCALE * np.cos(theta + MARGIN)
    S2 = S - np.exp(l_t) + np.exp(l_m)
    loss = np.mean(np.log(S2) - l_m)
    return np.float32(loss), res


def kernel(**inputs):
    loss, _ = run(inputs, trace=False)
    return np.asarray(loss, dtype=np.float32).reshape(())


# revision 17
# speedup vs baseline: 1.0799x; 1.0239x over previous
"""ArcFace loss on 8 TRN2 NeuronCores (Bass/Tile).

Strategy (model-parallel classification head, device kernel = pure
matmul+exp stream):
  - Host: l2-normalize embeddings and weights (fp32), quantize to fp8
    (e_hat*32, w_hat*128), shard classes across 8 cores (12500/core,
    zero-padded to 12544), and compute the per-row target-class
    corrections (ArcFace margin) in float64 from the fp32 inputs.
  - Device (per core): cosine slice = e_hat @ w_hat_local^T on the
    TensorEngine (fp8 DoubleRow), then sum_c exp(64*cos) per 2048-col
    PSUM group, spread over up to three drain engines so the PE never
    stalls (a stall-free PE stream also lets the clock ramp MID->MAX):
      * ACT: exp activation with accum_out (exact, scale 2^-6).
      * DVE/Pool: Schraudolph bf16 exp -- i16 = f32_to_i16(psm*a + b)
        is a piecewise-linear log2 approximation whose bit pattern IS
        exp(64*cos) in bf16; a second pass sums the bitcast values
        (DVE: tensor_scalar+accum, Pool: reduce_sum). ~0.3% rms /
        ~0.2% mean error per group sum, correctable on host (slots
        are output separately).
  - Host: S[b] = sum over cores/groups of the slot sums (minus the
    zero-pad columns that contribute exp(0)=1 each), swap in the margin
    target term, loss = mean(log(S') - l_m). No on-device collectives:
    the cross-core reduction is 8 x 28KB, cheaper on host than a ~29us
    mesh AllReduce.

kernel(**inputs) takes the FULL inputs and returns the full (scalar)
output.
"""

import math

import numpy as np
import ml_dtypes

import concourse.bass as bass
import concourse.mybir as mybir
import concourse.tile as tile
from concourse import bacc

AF = mybir.ActivationFunctionType
ALU = mybir.AluOpType
AX = mybir.AxisListType
F32 = mybir.dt.float32
BF16 = mybir.dt.bfloat16
I16 = mybir.dt.int16
FP8 = mybir.dt.float8e4

MARGIN = 0.5
SCALE = 64.0
EPS = 1e-7

S_E = 32.0
S_W = 128.0

# bf16 Schraudolph: bits(exp(t)) ~ 128*(t*log2(e) + 127) - C,  C ~= 7
A_SCH = 128.0 * math.log2(math.e) * (SCALE / (S_E * S_W))
B_SCH = 16256.0 - 7.0
# host-side divisor for the systematic Schraudolph bias on DVE/Pool slots
# (measured +0.22% on HW with round-to-nearest f32->i16 conversion)
SCH_BIAS = 0.0022

# drain-engine pattern over full-width (2048) tiles, cycled.
# 'A': ACT exp with accum_out (2.25us/tile, self-contained).
# 'E': ACT exp without accum (1.97us/tile) + DVE sums the bf16 output
#      tile (2.37us/tile) OFF the critical path -- the DVE read hits
#      SBUF scr, not PSUM, so it adds no PSUM-release coupling.
# 'D': full DVE Schraudolph (4.6us/tile) -- measured slower overall
#      (PSUM-coupling stall tax); kept for reference, unused.
# GPSIMD can neither read PSUM nor run reduce/accum ops.
DRAIN_PATTERN = "EEEEEA"


def make_cfg(n_cores=8, b=1024, d=512, c_total=100000, pattern=None):
    c_local = c_total // n_cores
    c_pad = ((c_local + 127) // 128) * 128
    grp_w = [2048, 2048, 2048, 2048, 2048, 2048, 256]
    assert sum(grp_w) == c_pad
    return dict(
        n_cores=n_cores,
        b=b,
        d=d,
        c_total=c_total,
        c_local=c_local,
        c_pad=c_pad,
        grp_w=grp_w,
        pattern=pattern or DRAIN_PATTERN,
    )


def _drains(cfg):
    """Map (gi, bo) -> 'A' | 'D' drain engine."""
    out = {}
    pat = cfg["pattern"]
    t = 0
    for gi, gw in enumerate(cfg["grp_w"]):
        for bo in range(cfg["b"] // 128):
            if gw < 2048:
                out[(gi, bo)] = "A"
            else:
                out[(gi, bo)] = pat[t % len(pat)]
                t += 1
    return out


def build_nc(cfg):
    n_cores = cfg["n_cores"]
    b, d = cfg["b"], cfg["d"]
    c_pad = cfg["c_pad"]
    grp_w = cfg["grp_w"]
    NG = len(grp_w)
    grp_off = [0]
    for gw in grp_w:
        grp_off.append(grp_off[-1] + gw)
    KO = d // 128
    BO = b // 128
    P = 128
    drains = _drains(cfg)

    nc = bacc.Bacc(
        "TRN2",
        target_bir_lowering=False,
        debug=False,
        enable_asserts=True,
        num_devices=n_cores,
    )

    wt_d = nc.dram_tensor("wt", [P, KO * c_pad], FP8, kind="ExternalInput")
    et_d = nc.dram_tensor("et", [P, KO * b], FP8, kind="ExternalInput")
    out_d = nc.dram_tensor("out", [P, BO * NG], F32, kind="ExternalOutput")

    with tile.TileContext(nc) as tc:
        with (
            tc.tile_pool(name="big", bufs=1) as pb,
            tc.tile_pool(name="wpool", bufs=NG) as pw,
            tc.tile_pool(name="scr", bufs=4) as pscr,
            tc.tile_pool(name="i16p", bufs=2) as pi16,
            tc.tile_pool(name="jnkp", bufs=2) as pjnk,
            tc.tile_pool(name="small", bufs=1) as ps,
            tc.tile_pool(name="ps_all", bufs=2, space="PSUM") as pps,
        ):
            # ---- load replicated embeddings (scalar queue: the ACT
            # engine is idle until the first exp, ~7us later). Split by
            # k-half so the kp=0 matmuls can start after 0.25MB lands ----
            et_sb = pb.tile([P, KO, b], FP8, tag="et")
            et_src = et_d.ap().rearrange("p (k b) -> p k b", k=KO)
            nc.scalar.dma_start(et_sb[:, 0:2, :], et_src[:, 0:2, :])
            nc.scalar.dma_start(et_sb[:, 2:4, :], et_src[:, 2:4, :])
            # ---- prefetch all weight groups across two DMA queues
            # (each group block is contiguous in DRAM) ----
            w_tiles = []
            for gi in range(NG):
                gw = grp_w[gi]
                c0 = grp_off[gi]
                Wg = pw.tile([P, KO, gw], FP8, tag=f"Wg{gi}", bufs=1)
                w_tiles.append(Wg)
                src = wt_d.ap()[:, KO * c0 : KO * (c0 + gw)].rearrange(
                    "p (k n) -> p k n", k=KO
                )
                if gi == 0:
                    # k-major block: each k-half is contiguous in DRAM,
                    # and the kp=0 matmuls only need the first half
                    nc.sync.dma_start(Wg[:, 0:2, :], src[:, 0:2, :])
                    nc.sync.dma_start(Wg[:, 2:4, :], src[:, 2:4, :])
                elif gi % 2 == 0:
                    nc.sync.dma_start(Wg[:, :, :gw], src)
                else:
                    nc.gpsimd.dma_start(Wg[:, :, :gw], src)

            sums = ps.tile([P, BO, NG], F32, tag="sums")
            for gi in range(NG):
                gw = grp_w[gi]
                Wg = w_tiles[gi]
                for bo in range(BO):
                    bs = slice(bo * P, (bo + 1) * P)
                    psm = pps.tile([P, 2048], F32, tag="ps")
                    for kp in range(KO // 2):
                        ks = slice(2 * kp, 2 * kp + 2)
                        for o in range(0, gw, 512):
                            nw = min(512, gw - o)
                            nc.tensor.matmul(
                                psm[:, o : o + nw],
                                et_sb[:, ks, bs],
                                Wg[:, ks, o : o + nw],
                                start=(kp == 0),
                                stop=(kp == KO // 2 - 1),
                                perf_mode=mybir.MatmulPerfMode.DoubleRow,
                            )
                    drain = drains[(gi, bo)]
                    slot = sums[:, bo, gi : gi + 1]
                    if drain == "A":
                        scr = pscr.tile([P, 2048], BF16, tag="escr")
                        nc.scalar.activation(
                            scr[:, :gw],
                            psm[:, :gw],
                            AF.Exp,
                            scale=SCALE / (S_E * S_W),
                            accum_out=slot,
                        )
                    elif drain == "E":
                        scr = pscr.tile([P, 2048], BF16, tag="escr")
                        nc.scalar.activation(
                            scr[:, :gw],
                            psm[:, :gw],
                            AF.Exp,
                            scale=SCALE / (S_E * S_W),
                        )
                        jnk = pjnk.tile([P, 2048], BF16, tag="jnk")
                        nc.vector.tensor_scalar(
                            jnk[:, :gw], scr[:, :gw],
                            1.0, 0.0, ALU.mult, ALU.add,
                            accum_out=slot,
                        )
                    elif drain == "D":
                        i16_t = pi16.tile([P, 2048], I16, tag="i16")
                        nc.vector.tensor_scalar(
                            i16_t[:, :gw], psm[:, :gw],
                            A_SCH, B_SCH, ALU.mult, ALU.add,
                        )
                        jnk = pjnk.tile([P, 2048], BF16, tag="jnk")
                        nc.vector.tensor_scalar(
                            jnk[:, :gw], i16_t[:, :gw].bitcast(BF16),
                            1.0, 0.0, ALU.mult, ALU.add,
                            accum_out=slot,
                        )
                    else:  # 'P'
                        i16_t = pi16.tile([P, 2048], I16, tag="i16g")
                        nc.gpsimd.tensor_scalar(
                            i16_t[:, :gw], psm[:, :gw],
                            A_SCH, B_SCH, ALU.mult, ALU.add,
                        )
                        nc.gpsimd.reduce_sum(
                            slot, i16_t[:, :gw].bitcast(BF16), axis=AX.X
                        )

            nc.sync.dma_start(
                out_d.ap(), sums[:].rearrange("p b g -> p (b g)")
            )

    nc.compile()
    return nc


def prep_inputs(cfg, embeddings, weight):
    """Normalize + quantize + shard the full inputs into per-core in_maps."""
    n_cores = cfg["n_cores"]
    b, d = cfg["b"], cfg["d"]
    c_local, c_pad = cfg["c_local"], cfg["c_pad"]
    KO = d // 128
    P = 128

    e = np.asarray(embeddings, np.float32)
    w = np.asarray(weight, np.float32)
    e_hat = e / np.maximum(
        np.linalg.norm(e, axis=-1, keepdims=True), 1e-12
    )
    w_hat = w / np.maximum(
        np.linalg.norm(w, axis=-1, keepdims=True), 1e-12
    )

    et = (e_hat.T * S_E).astype(ml_dtypes.float8_e4m3)
    et_host = np.ascontiguousarray(
        et.reshape(KO, P, b).transpose(1, 0, 2).reshape(P, KO * b)
    )

    in_maps = []
    for i in range(n_cores):
        ws = w_hat[i * c_local : (i + 1) * c_local]
        if c_pad > c_local:
            ws = np.concatenate(
                [ws, np.zeros((c_pad - c_local, d), np.float32)], axis=0
            )
        wt = (ws * S_W).astype(ml_dtypes.float8_e4m3).T  # [d, c_pad]
        wt4 = np.ascontiguousarray(wt).reshape(KO, P, c_pad)
        blocks = []
        c0 = 0
        for gw in cfg["grp_w"]:
            blk = wt4[:, :, c0 : c0 + gw]  # [KO, P, gw]
            blocks.append(blk.transpose(1, 0, 2).reshape(P, KO * gw))
            c0 += gw
        wt_host = np.ascontiguousarray(np.concatenate(blocks, axis=1))
        in_maps.append({"wt": wt_host, "et": et_host})
    return in_maps, e_hat, w_hat


_CACHED = {}


def _get_nc(cfg_key, cfg):
    if cfg_key not in _CACHED:
        _CACHED[cfg_key] = build_nc(cfg)
    return _CACHED[cfg_key]


def run(inputs, mm_dtype="fp8", trace=False, **kw):
    from concourse.bass_utils import run_bass_kernel_spmd

    cfg = make_cfg()
    nc = _get_nc((mm_dtype,), cfg)
    in_maps, e_hat, w_hat = prep_inputs(
        cfg, inputs["embeddings"], inputs["weight"]
    )
    res = run_bass_kernel_spmd(
        nc, in_maps, core_ids=list(range(cfg["n_cores"])), trace=trace, **kw
    )

    b = cfg["b"]
    BO = b // 128
    grp_w = cfg["grp_w"]
    NG = len(grp_w)
    drains = _drains(cfg)
    # per-slot correction for the Schraudolph bias on DVE/Pool groups
    corr = np.ones((BO, NG), np.float64)
    for gi in range(NG):
        for bo in range(BO):
            if drains[(gi, bo)] == "D":
                corr[bo, gi] = 1.0 / (1.0 + SCH_BIAS)
    # slot[p, bo, gi] holds rows b = bo*128 + p
    S = np.zeros(b, np.float64)
    for i in range(cfg["n_cores"]):
        slots = (
            res.results[i]["out"].astype(np.float64).reshape(128, BO, NG)
        )
        slots = slots * corr[None, :, :]
        S += slots.sum(axis=2).T.reshape(-1)
    # each core's (c_pad - c_local) zero-pad columns sit in the last
    # (ACT-drained) group and contribute exp(0) = 1 exactly
    S -= float(cfg["n_cores"] * (cfg["c_pad"] - cfg["c_local"]))

    labels = np.asarray(inputs["labels"]).astype(np.int64)
    cos_t = np.einsum(
        "bd,bd->b",
        e_hat.astype(np.float64),
        w_hat[labels].astype(np.float64),
    )
    cos_c = np.clip(cos_t, -1.0 + EPS, 1.0 - EPS)
    theta = np.arccos(cos_c)
    l_t = SCALE * cos_t
    l_m = SCALE * np.cos(theta + MARGIN)
    S2 = S - np.exp(l_t) + np.exp(l_m)
    loss = np.mean(np.log(S2) - l_m)
    return np.float32(loss), res


def kernel(**inputs):
    loss, _ = run(inputs, trace=False)
    return np.asarray(loss, dtype=np.float32).reshape(())
